# revision 27
# baseline (speedup 1.0000x reference)
"""Trainium2 Bass kernel for nn_CSATransformer_25778393710760.

Math: the reference module (eval mode) computes
    p   = softmax(wt(w1(x) + w2(c) + bsa), dim=-2);  h = x * p
    A   = softmax(mask_diag(sigmoid(si + sj^T)), -1); colsum = A.sum(1)
    ui  = x * colsum[..., None]
    y   = PFF(ui) + ui;  out = LN(y) * g + b
With the given parameters (all biases zero, ln identity), PFF is positively
homogeneous (relu(c*z) = c*relu(z) for c > 0) and colsum > 0, so
    y = diag(colsum) @ (x + PFF(x))
and LayerNorm cancels the positive per-row scale up to the eps term
(relative effect ~ eps/var * (1 - 1/colsum^2) ~ 1e-8).  Hence
    out = LN(relu(x @ pfn_w1) @ pfn_w2 + x) * ln_g + ln_b
to well below f32 noise (verified 4.5e-6 max rel err vs the f32 reference).

Sharding: pure data parallel over batch B=8 across the 8 NeuronCores.

v2.1 kernel layout per core (one batch example, L=4096 rows of D=128),
bf16 matmul path (max rel err ~4e-3 vs f32 reference, tol 2e-2):
8 slabs of 512 rows:
  f32 loads (slab 0/1 HWDGE for fast start, pairs on SWDGE) ->
  PE transpose to (d,l) -> ACT copy PSUM->SBUF (casts to bf16) ->
  PE w1 matmul (bf16) -> ACT relu -> PE w2 matmul (bf16) ->
  DVE residual add (y2p + xT) -> xbar DMA transpose back to natural
  (UNPADDED [128,c,128] out only -- padded strides scramble on HW) ->
  DVE bn_stats per chunk + slab-pair-batched even/odd combine ->
  normalize chunks split DVE/gpsimd/ACT -> paired f32 stores on sync.
"""

import os
import numpy as np

B, L, DX = 8, 4096, 128
_SLABS = 8          # 512-row slabs per core
_CPS = 4            # 128-row chunks per slab

_prog_cache = {}


def _build_program():
    import concourse.tile as tile
    from concourse import bacc, mybir
    from concourse.bass import ts

    f32 = mybir.dt.float32
    bf16 = mybir.dt.bfloat16
    AF = mybir.ActivationFunctionType
    OP = mybir.AluOpType

    nc = bacc.Bacc(None, target_bir_lowering=False)
    x = nc.dram_tensor("x", [L, DX], f32, kind="ExternalInput")
    w1 = nc.dram_tensor("w1", [DX, DX], bf16, kind="ExternalInput")
    w2 = nc.dram_tensor("w2", [DX, DX], bf16, kind="ExternalInput")
    identp = nc.dram_tensor("identp", [DX, DX + 1], f32, kind="ExternalInput")
    identb = nc.dram_tensor("identb", [DX, DX], bf16, kind="ExternalInput")
    y = nc.dram_tensor("y", [L, DX], f32, kind="ExternalOutput")

    with tile.TileContext(nc) as tc:
        with (
            tc.tile_pool(name="consts", bufs=1) as consts,
            tc.tile_pool(name="xg_pool", bufs=3) as xg_pool,
            tc.tile_pool(name="work", bufs=4) as work,
            tc.tile_pool(name="nat", bufs=4) as nat,
            tc.tile_pool(name="og_pool", bufs=3) as og_pool,
            tc.tile_pool(name="small", bufs=3) as small,
            tc.tile_pool(name="ps_t", bufs=3, space="PSUM") as ps_t,
            tc.tile_pool(name="ps_m1", bufs=2, space="PSUM") as ps_m1,
            tc.tile_pool(name="ps_m2", bufs=2, space="PSUM") as ps_m2,
            tc.tile_pool(name="ps_w", bufs=1, space="PSUM") as ps_w,
        ):
            # ---- tiny const DMAs first: transposes gate on ident ----
            identp_sb = consts.tile([128, 129], f32)
            nc.sync.dma_start(out=identp_sb, in_=identp[:, :])
            ident = identp_sb[:, 0:128]
            identb_sb = consts.tile([128, 128], bf16)
            nc.scalar.dma_start(out=identb_sb, in_=identb[:, :])

            # ---- loads: chunk c = rows congruent to c (mod 4), so each
            # partition's bytes are contiguous in HBM (2KB descriptors).
            # LayerNorm is row-wise, so the permutation flows through ----
            xgs = []
            xg0 = xg_pool.tile([128, _CPS, 128], f32, tag="xg0")
            nc.sync.dma_start(
                out=xg0, in_=x[ts(0, 512), :].rearrange("(p c) d -> p c d", c=_CPS)
            )
            xgs.append(xg0)
            w1_sb = consts.tile([128, 128], bf16)
            w2_sb = consts.tile([128, 128], bf16)
            nc.sync.dma_start(out=w1_sb, in_=w1[:, :])
            nc.scalar.dma_start(out=w2_sb, in_=w2[:, :])
            xg1 = xg_pool.tile([128, _CPS, 128], f32, tag="xg1")
            nc.scalar.dma_start(
                out=xg1, in_=x[ts(1, 512), :].rearrange("(p c) d -> p c d", c=_CPS)
            )
            xgs.append(xg1)
            # bulk loads ride the SWDGE ring so HWDGE stays free for the
            # transposes and stores
            for gg in range(1, 4):
                xg = xg_pool.tile([128, 2, _CPS, 128], f32, tag="xgd")
                src = x[ts(gg, 1024), :].rearrange(
                    "(g p c) d -> p g c d", g=2, c=_CPS
                )
                nc.gpsimd.dma_start(out=xg, in_=src)
                xgs.append(xg[:, 0, :, :])
                xgs.append(xg[:, 1, :, :])
            eps = consts.tile([128, 1], f32)
            nc.vector.memset(eps, 1e-6)

            # spin the PE on dummy bf16 transposes while waiting for x DMAs
            # (bf16: single-pass, so the spins don't block slab 0's work)
            pewarm = ps_w.tile([128, 128], bf16, tag="warm")
            for _ in range(8):
                nc.tensor.transpose(pewarm, identb_sb, identb_sb)
            warmsink = consts.tile([128, 1], bf16)
            nc.vector.tensor_copy(out=warmsink, in_=pewarm[:, 0:1])
            # warm the ACT tables off the critical path
            warm = consts.tile([128, 1], f32)
            nc.scalar.activation(out=warm, in_=eps, func=AF.Relu)
            nc.scalar.activation(out=warm, in_=eps, func=AF.Sqrt)
            nc.scalar.activation(out=warm, in_=eps, func=AF.Identity, bias=eps)

            ogs = []
            bsts = []
            stores = []
            for g in range(_SLABS):
                # ---- PE transpose to (d, l); xtp PSUM f32 ----
                xtp = ps_t.tile([128, _CPS, 128], f32, tag="xtp")
                for c in range(_CPS):
                    nc.tensor.transpose(xtp[:, c, :], xgs[g][:, c, :], ident)
                xT = work.tile([128, _CPS, 128], bf16, tag="xT")
                nc.scalar.copy(out=xT, in_=xtp)
                xT2 = xT.rearrange("p c d -> p (c d)")

                # ---- PFF: y1 = relu(w1T @ xT); y2 = w2T @ y1 ----
                y1p = ps_m1.tile([128, 512], f32, tag="mm1")
                nc.tensor.matmul(y1p, lhsT=w1_sb, rhs=xT2, start=True, stop=True)
                y1s = work.tile([128, 512], bf16, tag="y1s")
                nc.scalar.activation(out=y1s, in_=y1p, func=AF.Relu)
                y2p = ps_m2.tile([128, 512], f32, tag="mm2")
                nc.tensor.matmul(y2p, lhsT=w2_sb, rhs=y1s, start=True, stop=True)

                # ---- residual in transposed space (fused PSUM->SBUF) ----
                pt = work.tile([128, 512], bf16, tag="pt")
                nc.vector.tensor_add(out=pt, in0=y2p, in1=xT2)

                # keep the HAM busy-window fed while PE idles between slabs
                nc.tensor.transpose(pewarm, identb_sb, identb_sb)
                nc.tensor.transpose(pewarm, identb_sb, identb_sb)

                # ---- transpose back to natural via xbar DMA (sync ring) ----
                pn = nat.tile([128, _CPS, 128], bf16, tag="pn")
                nc.sync.dma_start_transpose(out=pn, in_=pt)

                # ---- LN stats per slab: bn_stats per chunk, manual even/odd
                # combine batched over the 4 chunks.  bn_stats 6-tuple:
                # (cnt_e, mean_e, cnt*var_e, cnt_o, mean_o, cnt*var_o);
                # mean=(m_e+m_o)/2, 128*var = cv_e+cv_o+32*(m_e-m_o)^2 ----
                bstats = small.tile([128, _CPS, 7], f32, tag="bstats")
                for c in range(_CPS):
                    nc.vector.bn_stats(out=bstats[:, c, 0:6], in_=pn[:, c, :])
                m_e, cv_e = bstats[:, :, 1], bstats[:, :, 2]
                m_o, cv_o = bstats[:, :, 4], bstats[:, :, 5]
                sm = small.tile([128, _CPS], f32, tag="sm")
                nc.vector.tensor_add(out=sm, in0=m_e, in1=m_o)
                dm = small.tile([128, _CPS], f32, tag="dm")
                nc.vector.tensor_tensor(out=dm, in0=m_e, in1=m_o, op=OP.subtract)
                vv = small.tile([128, _CPS], f32, tag="vv")
                nc.vector.tensor_add(out=vv, in0=cv_e, in1=cv_o)
                v2 = small.tile([128, _CPS], f32, tag="v2")
                nc.vector.scalar_tensor_tensor(
                    out=v2, in0=dm, scalar=32.0, in1=dm, op0=OP.mult, op1=OP.mult
                )
                v3 = small.tile([128, _CPS], f32, tag="v3")
                nc.vector.tensor_add(out=v3, in0=vv, in1=v2)
                std = small.tile([128, _CPS], f32, tag="std")
                nc.scalar.activation(
                    out=std, in_=v3, func=AF.Sqrt, scale=1.0 / 128.0, bias=eps
                )
                rstd = small.tile([128, _CPS], f32, tag="rstd")
                nc.vector.reciprocal(out=rstd, in_=std)
                nmr = small.tile([128, _CPS], f32, tag="nmr")
                nc.vector.scalar_tensor_tensor(
                    out=nmr, in0=sm, scalar=-0.5, in1=rstd,
                    op0=OP.mult, op1=OP.mult,
                )

                # ---- normalize this slab into its half of the pair tile ----
                if g % 2 == 0:
                    og = og_pool.tile([128, 2, _CPS, 128], f32, tag="og")
                    ogs.append(og)
                og = ogs[-1]
                engs = ["v", "g", "a", "g"]
                for c in range(_CPS):
                    r1 = rstd[:, c : c + 1]
                    n1 = nmr[:, c : c + 1]
                    src = pn[:, c, :]
                    dst_c = og[:, g % 2, c, :]
                    e = engs[c]
                    if e == "v":
                        nc.vector.tensor_scalar(
                            out=dst_c, in0=src,
                            scalar1=r1, scalar2=n1, op0=OP.mult, op1=OP.add,
                        )
                    elif e == "a":
                        nc.scalar.activation(
                            out=dst_c, in_=src, func=AF.Identity,
                            bias=n1, scale=r1,
                        )
                    else:
                        nc.gpsimd.tensor_scalar(
                            out=dst_c, in0=src,
                            scalar1=r1, scalar2=n1, op0=OP.mult, op1=OP.add,
                        )

                # ---- paired store on the SWDGE ring: the sync ring carries
                # only the transpose-backs, so they never queue behind a
                # store whose og isn't ready yet ----
                if g % 2 == 1:
                    dst = y[ts(g // 2, 1024), :].rearrange(
                        "(g p c) d -> p g c d", g=2, c=_CPS
                    )
                    nc.gpsimd.dma_start(out=dst, in_=og)
    nc.finalize()
    return nc


def _ensure_ntff_hook():
    """Register the axon NTFF profiling hook if the image lacks antenv.axon_hooks."""
    try:
        from antenv.axon_hooks import get_axon_ntff_profile_hook  # noqa: F401
        return
    except ImportError:
        pass
    import sys
    import types

    import antenv
    from trn_agent_boot.trn_boot import _ntff_profile_via_ctypes

    hook = _ntff_profile_via_ctypes("/opt/axon/libaxon_pjrt.so")
    mod = types.ModuleType("antenv.axon_hooks")
    mod._hook = hook
    mod.set_axon_ntff_profile_hook = lambda h: setattr(mod, "_hook", h)
    mod.get_axon_ntff_profile_hook = lambda: mod._hook
    sys.modules["antenv.axon_hooks"] = mod
    antenv.axon_hooks = mod


def _make_in_maps(x, w1, w2):
    import ml_dtypes

    bf = ml_dtypes.bfloat16
    w1b = np.ascontiguousarray(w1, dtype=np.float32).astype(bf)
    w2b = np.ascontiguousarray(w2, dtype=np.float32).astype(bf)
    identp = np.concatenate(
        [np.eye(DX, dtype=np.float32), np.ones((DX, 1), np.float32)], axis=1
    )
    identb = np.eye(DX, dtype=np.float32).astype(bf)
    return [
        {
            "x": np.ascontiguousarray(x[b], dtype=np.float32),
            "w1": w1b,
            "w2": w2b,
            "identp": identp,
            "identb": identb,
        }
        for b in range(B)
    ]


def _run_device(x, w1, w2, trace=False):
    import concourse.bass_utils as bass_utils
    from concourse.bass_utils import run_bass_kernel_spmd

    if trace:
        try:
            _ensure_ntff_hook()
            bass_utils.upload_artifacts = lambda tmpdir: str(tmpdir)
        except Exception as e:  # profiling is best-effort
            print(f"ntff hook unavailable ({e}); running without trace")
            trace = False

    if "prog" not in _prog_cache:
        _prog_cache["prog"] = _build_program()
    nc = _prog_cache["prog"]
    in_maps = _make_in_maps(x, w1, w2)
    res = run_bass_kernel_spmd(
        nc, in_maps, core_ids=list(range(B)), trace=trace,
        trace_cores=list(range(B)) if trace else None,
    )
    kernel.last_result = res
    kernel.last_exec_time_ns = res.exec_time_ns
    return np.stack([r["y"] for r in res.results], axis=0)


def _numpy_fallback(inputs):
    """Faithful (but slow) mirror of the reference for unexpected inputs."""
    f32 = np.float32
    x = np.asarray(inputs["x"], f32)
    c = np.asarray(inputs["c"], f32)
    W1 = np.asarray(inputs["W1"], f32); W2 = np.asarray(inputs["W2"], f32)
    wt_w = np.asarray(inputs["wt_w"], f32); bsa = np.asarray(inputs["bsa"], f32)
    Wsa1 = np.asarray(inputs["Wsa1"], f32); Wsa2 = np.asarray(inputs["Wsa2"], f32)
    wsat_w = np.asarray(inputs["wsat_w"], f32)
    wsat_b = np.asarray(inputs["wsat_b"], f32); bsa1 = np.asarray(inputs["bsa1"], f32)
    pfn_w1 = np.asarray(inputs["pfn_w1"], f32); pfn_b1 = np.asarray(inputs["pfn_b1"], f32)
    pfn_w2 = np.asarray(inputs["pfn_w2"], f32); pfn_b2 = np.asarray(inputs["pfn_b2"], f32)
    ln_g = np.asarray(inputs["ln_g"], f32); ln_b = np.asarray(inputs["ln_b"], f32)
    Bs, Ls, _ = x.shape
    wx = x @ W1
    wq = c @ W2
    logits = (wx + wq[:, None, :] + bsa) @ wt_w
    m = logits.max(-1, keepdims=True)
    e = np.exp(logits - m)
    p = (e / e.sum(-1, keepdims=True))[..., None]
    h = x * p
    si = (h @ Wsa1) @ wsat_w
    sj = (h @ Wsa2) @ wsat_w
    const = bsa1 @ wsat_w + wsat_b
    colsum = np.zeros((Bs, Ls), f32)
    blk = 512
    for b in range(Bs):
        for i0 in range(0, Ls, blk):
            s = 1.0 / (1.0 + np.exp(-(si[b, i0 : i0 + blk, None] + sj[b, None, :] + const)))
            for r in range(s.shape[0]):
                s[r, i0 + r] = -np.inf
            sm = s.max(-1, keepdims=True)
            ee = np.exp(s - sm)
            colsum[b] += (ee / ee.sum(-1, keepdims=True)).sum(0)
    ui = x * colsum[..., None]
    yv = np.maximum(ui @ pfn_w1 + pfn_b1, 0.0)
    yv = yv @ pfn_w2 + pfn_b2 + ui
    mu = yv.mean(-1, keepdims=True)
    var = ((yv - mu) ** 2).mean(-1, keepdims=True)
    return ((yv - mu) / np.sqrt(var + 1e-6) * ln_g + ln_b).astype(f32)


def kernel(**inputs):
    x = np.asarray(inputs["x"], dtype=np.float32)
    pfn_w1 = np.asarray(inputs["pfn_w1"], dtype=np.float32)
    pfn_w2 = np.asarray(inputs["pfn_w2"], dtype=np.float32)

    fast_ok = (
        x.shape == (B, L, DX)
        and not np.any(np.asarray(inputs["pfn_b1"]))
        and not np.any(np.asarray(inputs["pfn_b2"]))
        and np.all(np.asarray(inputs["ln_g"]) == 1.0)
        and not np.any(np.asarray(inputs["ln_b"]))
    )
    if not fast_ok:
        return _numpy_fallback(inputs)

    trace = bool(int(os.environ.get("CSA_TRACE", "0")))
    return _run_device(x, pfn_w1, pfn_w2, trace=trace)


kernel.last_exec_time_ns = None
kernel.last_result = None


# revision 35
# speedup vs baseline: 1.2387x; 1.2387x over previous
"""Trainium2 Bass kernel for nn_CSATransformer_25778393710760.

Math: the reference module (eval mode) computes
    p   = softmax(wt(w1(x) + w2(c) + bsa), dim=-2);  h = x * p
    A   = softmax(mask_diag(sigmoid(si + sj^T)), -1); colsum = A.sum(1)
    ui  = x * colsum[..., None]
    y   = PFF(ui) + ui;  out = LN(y) * g + b
With the given parameters (all biases zero, ln identity), PFF is positively
homogeneous (relu(c*z) = c*relu(z) for c > 0) and colsum > 0, so
    y = diag(colsum) @ (x + PFF(x))
and LayerNorm cancels the positive per-row scale up to the eps term
(relative effect ~ eps/var * (1 - 1/colsum^2) ~ 1e-8).  Hence
    out = LN(relu(x @ pfn_w1) @ pfn_w2 + x) * ln_g + ln_b
to well below f32 noise (verified 4.5e-6 max rel err vs the f32 reference,
identical to the reference's own f32-vs-f64 noise floor).

Sharding: pure data parallel over batch B=8 across the 8 NeuronCores.

Kernel layout per core (one batch example, L=4096 rows of D=128):
8 slabs of 512 rows, fully streaming:
  DMA in -> PE transpose to (d,l) -> w1 matmul + relu -> w2 matmul +
  residual add -> PE transpose back -> bn_stats/bn_aggr LN stats ->
  normalize (DVE/ACT split) -> DMA out.
DMA placement matters: slab-0 per-chunk on the two HWDGE rings, bulk
loads throttled (pool bufs) on the gpsimd SWDGE ring so they do not
steal SDMA bandwidth/queue service from the pipeline-filling loads.
"""

import os
import numpy as np

B, L, DX = 8, 4096, 128
_SLABS = 8          # 512-row slabs per core
_CPS = 4            # 128-row chunks per slab

_prog_cache = {}


def _build_program():
    import concourse.tile as tile
    from concourse import bacc, mybir
    from concourse.bass import ts

    f32 = mybir.dt.float32
    AF = mybir.ActivationFunctionType
    OP = mybir.AluOpType

    nc = bacc.Bacc(None, target_bir_lowering=False)
    bf16 = mybir.dt.bfloat16
    x = nc.dram_tensor("x", [L, DX], f32, kind="ExternalInput")
    w1 = nc.dram_tensor("w1", [DX, DX], bf16, kind="ExternalInput")
    w2 = nc.dram_tensor("w2", [DX, DX], bf16, kind="ExternalInput")
    identp = nc.dram_tensor("identp", [DX, DX + 1], f32, kind="ExternalInput")
    y = nc.dram_tensor("y", [L, DX], f32, kind="ExternalOutput")

    with tile.TileContext(nc) as tc:
        with (
            tc.tile_pool(name="consts", bufs=1) as consts,
            tc.tile_pool(name="io", bufs=3) as io,
            tc.tile_pool(name="work", bufs=3) as work,
            tc.tile_pool(name="small", bufs=4) as small,
            tc.tile_pool(name="xg_pool", bufs=4) as xg_pool,
            tc.tile_pool(name="ps_t", bufs=2, space="PSUM") as ps_t,
            tc.tile_pool(name="ps_mm", bufs=3, space="PSUM") as ps_mm,
            tc.tile_pool(name="ps_out", bufs=3, space="PSUM") as ps_out,
        ):
            # ---- tiny const DMAs first: transposes gate on ident ----
            identp_sb = consts.tile([128, 129], f32)
            nc.sync.dma_start(out=identp_sb, in_=identp[:, :])
            ident = identp_sb[:, 0:128]

            # ---- issue all x loads up front so slab 0 lands ASAP ----
            # chunk c = rows congruent to c (mod 4): each partition's bytes
            # are contiguous in HBM (2KB descriptors, ~3x faster loads).
            # LayerNorm is row-wise so the permutation flows through.
            xgs = []
            xg0 = xg_pool.tile([128, _CPS, 128], f32, tag="xg0")
            nc.sync.dma_start(
                out=xg0, in_=x[ts(0, 512), :].rearrange("(p c) d -> p c d", c=_CPS)
            )
            xgs.append(xg0)
            w1_sb = consts.tile([128, 128], bf16)
            w2_sb = consts.tile([128, 128], bf16)
            nc.sync.dma_start(out=w1_sb, in_=w1[:, :])
            nc.scalar.dma_start(out=w2_sb, in_=w2[:, :])
            xg1 = xg_pool.tile([128, _CPS, 128], f32, tag="xg1")
            nc.scalar.dma_start(
                out=xg1, in_=x[ts(1, 512), :].rearrange("(p c) d -> p c d", c=_CPS)
            )
            xgs.append(xg1)
            # bulk loads ride the idle gpsimd SWDGE ring so the sync/scalar
            # queues stay short (their EVSEMs gate the first transposes)
            for g in range(2, _SLABS):
                xg = xg_pool.tile([128, _CPS, 128], f32, tag="xg")
                src = x[ts(g, 512), :].rearrange("(p c) d -> p c d", c=_CPS)
                nc.gpsimd.dma_start(out=xg, in_=src)
                xgs.append(xg)
            w1_mm, w2_mm = w1_sb, w2_sb
            eps = consts.tile([128, 1], f32)
            nc.vector.memset(eps, 1e-6)
            # spin the PE on dummy transposes while waiting for x DMAs:
            # ~4us of sustained activity flips the HAM clock gate to 2.4GHz
            # before the real matmuls start (cold fp32 matmuls run at half
            # rate)
            pewarm = ps_t.tile([128, _CPS, 128], f32, tag="xtp")
            for _ in range(18):
                nc.tensor.transpose(pewarm[:, 0, :], ident, ident)
            warmsink = consts.tile([128, 1], f32)
            nc.vector.tensor_copy(out=warmsink, in_=pewarm[:, 0, 0:1])
            # warm up the ACT table sets off the critical path
            warm = consts.tile([128, 1], f32)
            nc.scalar.activation(out=warm, in_=eps, func=AF.Relu)
            nc.scalar.activation(out=warm, in_=eps, func=AF.Sqrt)
            nc.scalar.activation(out=warm, in_=eps, func=AF.Identity, bias=eps)

            for g in range(_SLABS):
                # ---- transpose to (d, l) layout ----
                xtp = ps_t.tile([128, _CPS, 128], f32, tag="xtp")
                for c in range(_CPS):
                    nc.tensor.transpose(xtp[:, c, :], xgs[g][:, c, :], ident)
                # xT in bf16: single-pass matmuls (fp32 is LOW_HIGH 2-pass)
                xT = work.tile([128, _CPS, 128], bf16, tag="xT")
                nc.scalar.copy(out=xT, in_=xtp)
                xT2 = xT.rearrange("p c d -> p (c d)")

                # ---- PFF: y1T = relu(w1T @ xT); PT = w2T @ y1T + xT ----
                y1p = ps_mm.tile([128, 512], f32, tag="mm")
                nc.tensor.matmul(y1p, lhsT=w1_mm, rhs=xT2, start=True, stop=True)
                y1s = work.tile([128, 512], bf16, tag="y1s")
                nc.scalar.activation(out=y1s, in_=y1p, func=AF.Relu)
                pp = ps_mm.tile([128, 512], f32, tag="mm")
                nc.tensor.matmul(pp, lhsT=w2_mm, rhs=y1s, start=True, stop=True)
                pt = work.tile([128, 512], f32, tag="pt")
                nc.vector.tensor_add(out=pt, in0=pp, in1=xT2)

                # ---- transpose back to (l, d) layout ----
                pn = ps_out.tile([128, _CPS, 128], f32, tag="pn")
                for c in range(_CPS):
                    nc.tensor.transpose(pn[:, c, :], pt[:, ts(c, 128)], ident)

                # ---- LN stats via bn_stats/bn_aggr per chunk ----
                bstats = small.tile([128, _CPS, 6], f32, tag="bstats")
                for c in range(_CPS):
                    nc.vector.bn_stats(out=bstats[:, c, :], in_=pn[:, c, :])
                mv = small.tile([128, _CPS, 2], f32, tag="mv")
                for c in range(_CPS):
                    nc.vector.bn_aggr(out=mv[:, c, :], in_=bstats[:, c, :])

                # rstd = 1/sqrt(var + eps); nmr = -mean * rstd
                # per-half so chunks 0-1 can normalize before 2-3 aggregate
                std = small.tile([128, _CPS], f32, tag="std")
                rstd = small.tile([128, _CPS], f32, tag="rstd")
                nmr = small.tile([128, _CPS], f32, tag="nmr")
                for hh in range(2):
                    hsl = slice(2 * hh, 2 * hh + 2)
                    nc.scalar.activation(
                        out=std[:, hsl], in_=mv[:, hsl, 1], func=AF.Sqrt,
                        scale=1.0, bias=eps,
                    )
                    nc.vector.reciprocal(out=rstd[:, hsl], in_=std[:, hsl])
                    nc.vector.scalar_tensor_tensor(
                        out=nmr[:, hsl], in0=mv[:, hsl, 0], scalar=-1.0,
                        in1=rstd[:, hsl], op0=OP.mult, op1=OP.mult,
                    )

                # ---- apply LN from PSUM: out = pn * rstd + nmr ----
                og = io.tile([128, _CPS, 128], f32, tag="og")
                for c in range(_CPS):
                    if c % 2 == 0:
                        nc.vector.tensor_scalar(
                            out=og[:, c, :], in0=pn[:, c, :],
                            scalar1=rstd[:, c : c + 1], scalar2=nmr[:, c : c + 1],
                            op0=OP.mult, op1=OP.add,
                        )
                    else:
                        nc.scalar.activation(
                            out=og[:, c, :], in_=pn[:, c, :], func=AF.Identity,
                            bias=nmr[:, c : c + 1], scale=rstd[:, c : c + 1],
                        )

                dst = y[ts(g, 512), :].rearrange("(p c) d -> p c d", c=_CPS)
                nc.sync.dma_start(out=dst, in_=og)
    nc.finalize()
    return nc


def _ensure_ntff_hook():
    """Register the axon NTFF profiling hook if the image lacks antenv.axon_hooks."""
    try:
        from antenv.axon_hooks import get_axon_ntff_profile_hook  # noqa: F401
        return
    except ImportError:
        pass
    import sys
    import types

    import antenv
    from trn_agent_boot.trn_boot import _ntff_profile_via_ctypes

    hook = _ntff_profile_via_ctypes("/opt/axon/libaxon_pjrt.so")
    mod = types.ModuleType("antenv.axon_hooks")
    mod._hook = hook
    mod.set_axon_ntff_profile_hook = lambda h: setattr(mod, "_hook", h)
    mod.get_axon_ntff_profile_hook = lambda: mod._hook
    sys.modules["antenv.axon_hooks"] = mod
    antenv.axon_hooks = mod


def _run_device(x, w1, w2, trace=False):
    import concourse.bass_utils as bass_utils
    from concourse.bass_utils import run_bass_kernel_spmd

    if trace:
        try:
            _ensure_ntff_hook()
            bass_utils.upload_artifacts = lambda tmpdir: str(tmpdir)
        except Exception as e:  # profiling is best-effort
            print(f"ntff hook unavailable ({e}); running without trace")
            trace = False

    import ml_dtypes

    if "prog" not in _prog_cache:
        _prog_cache["prog"] = _build_program()
    nc = _prog_cache["prog"]
    bf = ml_dtypes.bfloat16
    w1c = np.ascontiguousarray(w1, dtype=np.float32).astype(bf)
    w2c = np.ascontiguousarray(w2, dtype=np.float32).astype(bf)
    identp = np.concatenate(
        [np.eye(DX, dtype=np.float32), np.ones((DX, 1), np.float32)], axis=1
    )
    in_maps = [
        {
            "x": np.ascontiguousarray(x[b], dtype=np.float32),
            "w1": w1c,
            "w2": w2c,
            "identp": identp,
        }
        for b in range(B)
    ]
    res = run_bass_kernel_spmd(
        nc, in_maps, core_ids=list(range(B)), trace=trace,
        trace_cores=list(range(B)) if trace else None,
    )
    kernel.last_result = res
    kernel.last_exec_time_ns = res.exec_time_ns
    return np.stack([r["y"] for r in res.results], axis=0)


def _numpy_fallback(inputs):
    """Faithful (but slow) mirror of the reference for unexpected inputs."""
    f32 = np.float32
    x = np.asarray(inputs["x"], f32)
    c = np.asarray(inputs["c"], f32)
    W1 = np.asarray(inputs["W1"], f32); W2 = np.asarray(inputs["W2"], f32)
    wt_w = np.asarray(inputs["wt_w"], f32); bsa = np.asarray(inputs["bsa"], f32)
    Wsa1 = np.asarray(inputs["Wsa1"], f32); Wsa2 = np.asarray(inputs["Wsa2"], f32)
    wsat_w = np.asarray(inputs["wsat_w"], f32)
    wsat_b = np.asarray(inputs["wsat_b"], f32); bsa1 = np.asarray(inputs["bsa1"], f32)
    pfn_w1 = np.asarray(inputs["pfn_w1"], f32); pfn_b1 = np.asarray(inputs["pfn_b1"], f32)
    pfn_w2 = np.asarray(inputs["pfn_w2"], f32); pfn_b2 = np.asarray(inputs["pfn_b2"], f32)
    ln_g = np.asarray(inputs["ln_g"], f32); ln_b = np.asarray(inputs["ln_b"], f32)
    Bs, Ls, _ = x.shape
    wx = x @ W1
    wq = c @ W2
    logits = (wx + wq[:, None, :] + bsa) @ wt_w
    m = logits.max(-1, keepdims=True)
    e = np.exp(logits - m)
    p = (e / e.sum(-1, keepdims=True))[..., None]
    h = x * p
    si = (h @ Wsa1) @ wsat_w
    sj = (h @ Wsa2) @ wsat_w
    const = bsa1 @ wsat_w + wsat_b
    colsum = np.zeros((Bs, Ls), f32)
    blk = 512
    for b in range(Bs):
        for i0 in range(0, Ls, blk):
            s = 1.0 / (1.0 + np.exp(-(si[b, i0 : i0 + blk, None] + sj[b, None, :] + const)))
            for r in range(s.shape[0]):
                s[r, i0 + r] = -np.inf
            sm = s.max(-1, keepdims=True)
            ee = np.exp(s - sm)
            colsum[b] += (ee / ee.sum(-1, keepdims=True)).sum(0)
    ui = x * colsum[..., None]
    yv = np.maximum(ui @ pfn_w1 + pfn_b1, 0.0)
    yv = yv @ pfn_w2 + pfn_b2 + ui
    mu = yv.mean(-1, keepdims=True)
    var = ((yv - mu) ** 2).mean(-1, keepdims=True)
    return ((yv - mu) / np.sqrt(var + 1e-6) * ln_g + ln_b).astype(f32)


def kernel(**inputs):
    x = np.asarray(inputs["x"], dtype=np.float32)
    pfn_w1 = np.asarray(inputs["pfn_w1"], dtype=np.float32)
    pfn_w2 = np.asarray(inputs["pfn_w2"], dtype=np.float32)

    fast_ok = (
        x.shape == (B, L, DX)
        and not np.any(np.asarray(inputs["pfn_b1"]))
        and not np.any(np.asarray(inputs["pfn_b2"]))
        and np.all(np.asarray(inputs["ln_g"]) == 1.0)
        and not np.any(np.asarray(inputs["ln_b"]))
    )
    if not fast_ok:
        return _numpy_fallback(inputs)

    trace = bool(int(os.environ.get("CSA_TRACE", "0")))
    return _run_device(x, pfn_w1, pfn_w2, trace=trace)


kernel.last_exec_time_ns = None
kernel.last_result = None


# revision 37
# speedup vs baseline: 1.2451x; 1.0052x over previous
"""Trainium2 Bass kernel for nn_CSATransformer_25778393710760.

Math: the reference module (eval mode) computes
    p   = softmax(wt(w1(x) + w2(c) + bsa), dim=-2);  h = x * p
    A   = softmax(mask_diag(sigmoid(si + sj^T)), -1); colsum = A.sum(1)
    ui  = x * colsum[..., None]
    y   = PFF(ui) + ui;  out = LN(y) * g + b
With the given parameters (all biases zero, ln identity), PFF is positively
homogeneous (relu(c*z) = c*relu(z) for c > 0) and colsum > 0, so
    y = diag(colsum) @ (x + PFF(x))
and LayerNorm cancels the positive per-row scale up to the eps term
(relative effect ~ eps/var * (1 - 1/colsum^2) ~ 1e-8).  Hence
    out = LN(relu(x @ pfn_w1) @ pfn_w2 + x) * ln_g + ln_b
to well below f32 noise (verified 4.5e-6 max rel err vs the f32 reference,
identical to the reference's own f32-vs-f64 noise floor).

Sharding: pure data parallel over batch B=8 across the 8 NeuronCores.

Kernel layout per core (one batch example, L=4096 rows of D=128):
8 slabs of 512 rows, fully streaming:
  DMA in -> PE transpose to (d,l) -> w1 matmul + relu -> w2 matmul +
  residual add -> PE transpose back -> bn_stats/bn_aggr LN stats ->
  normalize (DVE/ACT split) -> DMA out.
DMA placement matters: slab-0 per-chunk on the two HWDGE rings, bulk
loads throttled (pool bufs) on the gpsimd SWDGE ring so they do not
steal SDMA bandwidth/queue service from the pipeline-filling loads.
"""

import os
import numpy as np

B, L, DX = 8, 4096, 128
_SLABS = 8          # 512-row slabs per core
_CPS = 4            # 128-row chunks per slab

_prog_cache = {}


def _build_program():
    import concourse.tile as tile
    from concourse import bacc, mybir
    from concourse.bass import ts

    f32 = mybir.dt.float32
    AF = mybir.ActivationFunctionType
    OP = mybir.AluOpType

    nc = bacc.Bacc(None, target_bir_lowering=False)
    bf16 = mybir.dt.bfloat16
    x = nc.dram_tensor("x", [L, DX], f32, kind="ExternalInput")
    w1 = nc.dram_tensor("w1", [DX, DX], bf16, kind="ExternalInput")
    w2 = nc.dram_tensor("w2", [DX, DX], bf16, kind="ExternalInput")
    identp = nc.dram_tensor("identp", [DX, DX + 1], f32, kind="ExternalInput")
    y = nc.dram_tensor("y", [L, DX], f32, kind="ExternalOutput")

    with tile.TileContext(nc) as tc:
        with (
            tc.tile_pool(name="consts", bufs=1) as consts,
            tc.tile_pool(name="io", bufs=3) as io,
            tc.tile_pool(name="work", bufs=3) as work,
            tc.tile_pool(name="small", bufs=4) as small,
            tc.tile_pool(name="xg_pool", bufs=4) as xg_pool,
            tc.tile_pool(name="ps_t", bufs=2, space="PSUM") as ps_t,
            tc.tile_pool(name="ps_mm", bufs=3, space="PSUM") as ps_mm,
            tc.tile_pool(name="ps_out", bufs=3, space="PSUM") as ps_out,
        ):
            # ---- tiny const DMAs first: transposes gate on ident ----
            identp_sb = consts.tile([128, 129], f32)
            nc.sync.dma_start(out=identp_sb, in_=identp[:, :])
            ident = identp_sb[:, 0:128]
            # bf16 identity for the bf16 transpose-back path
            identb = consts.tile([128, 128], bf16)
            nc.scalar.copy(out=identb, in_=ident)

            # ---- issue all x loads up front so slab 0 lands ASAP ----
            # chunk c = rows congruent to c (mod 4): each partition's bytes
            # are contiguous in HBM (2KB descriptors, ~3x faster loads).
            # LayerNorm is row-wise so the permutation flows through.
            xgs = []
            xg0 = xg_pool.tile([128, _CPS, 128], f32, tag="xg0")
            nc.sync.dma_start(
                out=xg0, in_=x[ts(0, 512), :].rearrange("(p c) d -> p c d", c=_CPS)
            )
            xgs.append(xg0)
            w1_sb = consts.tile([128, 128], bf16)
            w2_sb = consts.tile([128, 128], bf16)
            nc.sync.dma_start(out=w1_sb, in_=w1[:, :])
            nc.scalar.dma_start(out=w2_sb, in_=w2[:, :])
            xg1 = xg_pool.tile([128, _CPS, 128], f32, tag="xg1")
            nc.scalar.dma_start(
                out=xg1, in_=x[ts(1, 512), :].rearrange("(p c) d -> p c d", c=_CPS)
            )
            xgs.append(xg1)
            # bulk loads ride the idle gpsimd SWDGE ring so the sync/scalar
            # queues stay short (their EVSEMs gate the first transposes)
            for g in range(2, _SLABS):
                xg = xg_pool.tile([128, _CPS, 128], f32, tag="xg")
                src = x[ts(g, 512), :].rearrange("(p c) d -> p c d", c=_CPS)
                nc.gpsimd.dma_start(out=xg, in_=src)
                xgs.append(xg)
            w1_mm, w2_mm = w1_sb, w2_sb
            eps = consts.tile([128, 1], f32)
            nc.vector.memset(eps, 1e-6)
            # spin the PE on dummy transposes while waiting for x DMAs:
            # ~4us of sustained activity flips the HAM clock gate to 2.4GHz
            # before the real matmuls start (cold fp32 matmuls run at half
            # rate)
            pewarm = ps_t.tile([128, _CPS, 128], f32, tag="xtp")
            for _ in range(18):
                nc.tensor.transpose(pewarm[:, 0, :], ident, ident)
            warmsink = consts.tile([128, 1], f32)
            nc.vector.tensor_copy(out=warmsink, in_=pewarm[:, 0, 0:1])
            # warm up the ACT table sets off the critical path
            warm = consts.tile([128, 1], f32)
            nc.scalar.activation(out=warm, in_=eps, func=AF.Relu)
            nc.scalar.activation(out=warm, in_=eps, func=AF.Sqrt)
            nc.scalar.activation(out=warm, in_=eps, func=AF.Identity, bias=eps)

            for g in range(_SLABS):
                # ---- transpose to (d, l) layout ----
                xtp = ps_t.tile([128, _CPS, 128], f32, tag="xtp")
                for c in range(_CPS):
                    nc.tensor.transpose(xtp[:, c, :], xgs[g][:, c, :], ident)
                # xT in bf16: single-pass matmuls (fp32 is LOW_HIGH 2-pass)
                xT = work.tile([128, _CPS, 128], bf16, tag="xT")
                nc.scalar.copy(out=xT, in_=xtp)
                xT2 = xT.rearrange("p c d -> p (c d)")

                # ---- PFF: y1T = relu(w1T @ xT); PT = w2T @ y1T + xT ----
                y1p = ps_mm.tile([128, 512], f32, tag="mm")
                nc.tensor.matmul(y1p, lhsT=w1_mm, rhs=xT2, start=True, stop=True)
                y1s = work.tile([128, 512], bf16, tag="y1s")
                nc.scalar.activation(out=y1s, in_=y1p, func=AF.Relu)
                pp = ps_mm.tile([128, 512], f32, tag="mm")
                nc.tensor.matmul(pp, lhsT=w2_mm, rhs=y1s, start=True, stop=True)
                # pt in bf16: transpose-back is single-pass (f32 is 2-pass)
                pt = work.tile([128, 512], bf16, tag="pt")
                nc.vector.tensor_add(out=pt, in0=pp, in1=xT2)

                # ---- transpose back to (l, d) layout (pn PSUM bf16) ----
                pn = ps_out.tile([128, _CPS, 128], bf16, tag="pn")
                for c in range(_CPS):
                    nc.tensor.transpose(pn[:, c, :], pt[:, ts(c, 128)], identb)

                # ---- LN stats via bn_stats/bn_aggr per chunk ----
                bstats = small.tile([128, _CPS, 6], f32, tag="bstats")
                for c in range(_CPS):
                    nc.vector.bn_stats(out=bstats[:, c, :], in_=pn[:, c, :])
                mv = small.tile([128, _CPS, 2], f32, tag="mv")
                for c in range(_CPS):
                    nc.vector.bn_aggr(out=mv[:, c, :], in_=bstats[:, c, :])

                # rstd = 1/sqrt(var + eps); nmr = -mean * rstd
                # per-half so chunks 0-1 can normalize before 2-3 aggregate
                std = small.tile([128, _CPS], f32, tag="std")
                rstd = small.tile([128, _CPS], f32, tag="rstd")
                nmr = small.tile([128, _CPS], f32, tag="nmr")
                for hh in range(2):
                    hsl = slice(2 * hh, 2 * hh + 2)
                    nc.scalar.activation(
                        out=std[:, hsl], in_=mv[:, hsl, 1], func=AF.Sqrt,
                        scale=1.0, bias=eps,
                    )
                    nc.vector.reciprocal(out=rstd[:, hsl], in_=std[:, hsl])
                    nc.vector.scalar_tensor_tensor(
                        out=nmr[:, hsl], in0=mv[:, hsl, 0], scalar=-1.0,
                        in1=rstd[:, hsl], op0=OP.mult, op1=OP.mult,
                    )

                # ---- apply LN from PSUM: out = pn * rstd + nmr ----
                og = io.tile([128, _CPS, 128], f32, tag="og")
                for c in range(_CPS):
                    if c % 2 == 0:
                        nc.vector.tensor_scalar(
                            out=og[:, c, :], in0=pn[:, c, :],
                            scalar1=rstd[:, c : c + 1], scalar2=nmr[:, c : c + 1],
                            op0=OP.mult, op1=OP.add,
                        )
                    else:
                        nc.scalar.activation(
                            out=og[:, c, :], in_=pn[:, c, :], func=AF.Identity,
                            bias=nmr[:, c : c + 1], scale=rstd[:, c : c + 1],
                        )

                dst = y[ts(g, 512), :].rearrange("(p c) d -> p c d", c=_CPS)
                nc.sync.dma_start(out=dst, in_=og)
    nc.finalize()
    return nc


def _ensure_ntff_hook():
    """Register the axon NTFF profiling hook if the image lacks antenv.axon_hooks."""
    try:
        from antenv.axon_hooks import get_axon_ntff_profile_hook  # noqa: F401
        return
    except ImportError:
        pass
    import sys
    import types

    import antenv
    from trn_agent_boot.trn_boot import _ntff_profile_via_ctypes

    hook = _ntff_profile_via_ctypes("/opt/axon/libaxon_pjrt.so")
    mod = types.ModuleType("antenv.axon_hooks")
    mod._hook = hook
    mod.set_axon_ntff_profile_hook = lambda h: setattr(mod, "_hook", h)
    mod.get_axon_ntff_profile_hook = lambda: mod._hook
    sys.modules["antenv.axon_hooks"] = mod
    antenv.axon_hooks = mod


def _run_device(x, w1, w2, trace=False):
    import concourse.bass_utils as bass_utils
    from concourse.bass_utils import run_bass_kernel_spmd

    if trace:
        try:
            _ensure_ntff_hook()
            bass_utils.upload_artifacts = lambda tmpdir: str(tmpdir)
        except Exception as e:  # profiling is best-effort
            print(f"ntff hook unavailable ({e}); running without trace")
            trace = False

    import ml_dtypes

    if "prog" not in _prog_cache:
        _prog_cache["prog"] = _build_program()
    nc = _prog_cache["prog"]
    bf = ml_dtypes.bfloat16
    w1c = np.ascontiguousarray(w1, dtype=np.float32).astype(bf)
    w2c = np.ascontiguousarray(w2, dtype=np.float32).astype(bf)
    identp = np.concatenate(
        [np.eye(DX, dtype=np.float32), np.ones((DX, 1), np.float32)], axis=1
    )
    in_maps = [
        {
            "x": np.ascontiguousarray(x[b], dtype=np.float32),
            "w1": w1c,
            "w2": w2c,
            "identp": identp,
        }
        for b in range(B)
    ]
    res = run_bass_kernel_spmd(
        nc, in_maps, core_ids=list(range(B)), trace=trace,
        trace_cores=list(range(B)) if trace else None,
    )
    kernel.last_result = res
    kernel.last_exec_time_ns = res.exec_time_ns
    return np.stack([r["y"] for r in res.results], axis=0)


def _numpy_fallback(inputs):
    """Faithful (but slow) mirror of the reference for unexpected inputs."""
    f32 = np.float32
    x = np.asarray(inputs["x"], f32)
    c = np.asarray(inputs["c"], f32)
    W1 = np.asarray(inputs["W1"], f32); W2 = np.asarray(inputs["W2"], f32)
    wt_w = np.asarray(inputs["wt_w"], f32); bsa = np.asarray(inputs["bsa"], f32)
    Wsa1 = np.asarray(inputs["Wsa1"], f32); Wsa2 = np.asarray(inputs["Wsa2"], f32)
    wsat_w = np.asarray(inputs["wsat_w"], f32)
    wsat_b = np.asarray(inputs["wsat_b"], f32); bsa1 = np.asarray(inputs["bsa1"], f32)
    pfn_w1 = np.asarray(inputs["pfn_w1"], f32); pfn_b1 = np.asarray(inputs["pfn_b1"], f32)
    pfn_w2 = np.asarray(inputs["pfn_w2"], f32); pfn_b2 = np.asarray(inputs["pfn_b2"], f32)
    ln_g = np.asarray(inputs["ln_g"], f32); ln_b = np.asarray(inputs["ln_b"], f32)
    Bs, Ls, _ = x.shape
    wx = x @ W1
    wq = c @ W2
    logits = (wx + wq[:, None, :] + bsa) @ wt_w
    m = logits.max(-1, keepdims=True)
    e = np.exp(logits - m)
    p = (e / e.sum(-1, keepdims=True))[..., None]
    h = x * p
    si = (h @ Wsa1) @ wsat_w
    sj = (h @ Wsa2) @ wsat_w
    const = bsa1 @ wsat_w + wsat_b
    colsum = np.zeros((Bs, Ls), f32)
    blk = 512
    for b in range(Bs):
        for i0 in range(0, Ls, blk):
            s = 1.0 / (1.0 + np.exp(-(si[b, i0 : i0 + blk, None] + sj[b, None, :] + const)))
            for r in range(s.shape[0]):
                s[r, i0 + r] = -np.inf
            sm = s.max(-1, keepdims=True)
            ee = np.exp(s - sm)
            colsum[b] += (ee / ee.sum(-1, keepdims=True)).sum(0)
    ui = x * colsum[..., None]
    yv = np.maximum(ui @ pfn_w1 + pfn_b1, 0.0)
    yv = yv @ pfn_w2 + pfn_b2 + ui
    mu = yv.mean(-1, keepdims=True)
    var = ((yv - mu) ** 2).mean(-1, keepdims=True)
    return ((yv - mu) / np.sqrt(var + 1e-6) * ln_g + ln_b).astype(f32)


def kernel(**inputs):
    x = np.asarray(inputs["x"], dtype=np.float32)
    pfn_w1 = np.asarray(inputs["pfn_w1"], dtype=np.float32)
    pfn_w2 = np.asarray(inputs["pfn_w2"], dtype=np.float32)

    fast_ok = (
        x.shape == (B, L, DX)
        and not np.any(np.asarray(inputs["pfn_b1"]))
        and not np.any(np.asarray(inputs["pfn_b2"]))
        and np.all(np.asarray(inputs["ln_g"]) == 1.0)
        and not np.any(np.asarray(inputs["ln_b"]))
    )
    if not fast_ok:
        return _numpy_fallback(inputs)

    trace = bool(int(os.environ.get("CSA_TRACE", "0")))
    return _run_device(x, pfn_w1, pfn_w2, trace=trace)


kernel.last_exec_time_ns = None
kernel.last_result = None
